# revision 24
# baseline (speedup 1.0000x reference)
"""Trainium2 Bass kernel for MultiHeadLatentAttention (B=2, S=2048, H=2048,
NH=16, HD=128, LAT=512), SPMD across 8 NeuronCores.

Sharding: 8 cores = 2 (batch) x 4 (head-group TP). Core c handles batch c//4
and head group j = c%4 = heads {j, 4+j, 8+j, 12+j} (slots 0..3). Heads j/4+j
are the raw x1/x2 slices of q_half; heads 8+j/12+j are their RoPE
combinations, so the up-projection shards 4-way with no duplication. Each
core computes its partial o_proj output (f32); the host sums the 4 partials
per batch and adds bo.

v3 structure:
- q path host-fused: Fq = Wq_up[core rows] @ Wq_down -> contract hidden
  directly into q slots 0/1 (kills the 4x-replicated q_down work).
- kv_lat f16; k_up runs inside the D loop per s-chunk (cheap, 4 matmuls per
  chain); ropes inline on DVE.
- phase A is a software-pipelined instruction stream: QK pairs (paced by the
  ACT exp chain) interleave with the PREVIOUS head's AV accumulation,
  denominator adds on DVE, and "fill" PE work (v-up chains first, then the
  previous q-chunk's o_proj) so the tensor engine never idles waiting for
  exps. o_proj results go psum -> DRAM f32 directly (no DVE copy).
"""
import os
import sys
import types
from contextlib import ExitStack

import numpy as np

if "/opt/trn_rl_repo" not in sys.path:
    sys.path.insert(0, "/opt/trn_rl_repo")

# ---------------------------------------------------------------------------
# NTFF-profile shim: antenv.axon_hooks is missing in this image; register a
# hook backed by the axon PJRT .so so trace=True can capture HW exec time.
# ---------------------------------------------------------------------------


def _install_axon_hooks_shim():
    if "antenv.axon_hooks" in sys.modules:
        return
    try:
        import antenv
        from trn_agent_boot.trn_boot import _ntff_profile_via_ctypes
        hook = _ntff_profile_via_ctypes("/opt/axon/libaxon_pjrt.so")
    except Exception:
        return
    mod = types.ModuleType("antenv.axon_hooks")
    mod.get_axon_ntff_profile_hook = lambda: hook
    mod.set_axon_ntff_profile_hook = lambda h: None
    sys.modules["antenv.axon_hooks"] = mod
    antenv.axon_hooks = mod


_install_axon_hooks_shim()

import concourse.bass as bass  # noqa: E402
import concourse.mybir as mybir  # noqa: E402
import concourse.tile as tile  # noqa: E402
from concourse import bacc  # noqa: E402
from concourse.bass_utils import run_bass_kernel_spmd  # noqa: E402

P = 128
H = 2048
NH = 16
HD = 128
LAT = 512
B = 2
S = 2048
ROPE_DIM = H // 4
NHG = 4          # heads per core
SC = 512         # s/q chunk (one PSUM bank of fp32)
INV_SQRT_HD = 0.08838834764831845  # 1/sqrt(128)

f32 = mybir.dt.float32
f16 = mybir.dt.float16
Act = mybir.ActivationFunctionType
Alu = mybir.AluOpType
F16 = np.float16


def build_mla(seq=S, debug=False):
    """Build one core's program. All cores run this same program SPMD."""
    NSC = seq // SC   # s-chunks
    HT = H // P       # 16 h-tiles
    LT = LAT // P     # 4 l-tiles
    ST = seq // P     # s-tiles (= k-tiles in attention)

    nc = bacc.Bacc("TRN2", target_bir_lowering=False, debug=debug)

    hsT = nc.dram_tensor("hsT", [H, seq], f16, kind="ExternalInput")
    FqT = nc.dram_tensor("FqT", [H, 2 * P], f16, kind="ExternalInput")
    bqf = nc.dram_tensor("bqf", [P, 2], f32, kind="ExternalInput")
    WkvdT = nc.dram_tensor("WkvdT", [H, LAT], f16, kind="ExternalInput")
    bkvd = nc.dram_tensor("bkvd", [LAT], f32, kind="ExternalInput")
    WkuT = nc.dram_tensor("WkuT", [LAT, 2 * P], f16, kind="ExternalInput")
    bku = nc.dram_tensor("bku", [P, 2], f32, kind="ExternalInput")
    WvuT = nc.dram_tensor("WvuT", [LAT, NHG * P], f16, kind="ExternalInput")
    bvu = nc.dram_tensor("bvu", [1, NHG * P], f32, kind="ExternalInput")
    WoT = nc.dram_tensor("WoT", [NHG * P, H], f16, kind="ExternalInput")
    cosT = nc.dram_tensor("cosT", [P, seq], f16, kind="ExternalInput")
    sinT = nc.dram_tensor("sinT", [P, seq], f16, kind="ExternalInput")
    ones = nc.dram_tensor("ones", [P, P], f16, kind="ExternalInput")
    outT = nc.dram_tensor("outT", [H, seq], f16, kind="ExternalOutput")

    with tile.TileContext(nc) as tc, ExitStack() as top:
        const = top.enter_context(tc.tile_pool(name="const", bufs=1))
        ao_pool = top.enter_context(tc.tile_pool(name="ao", bufs=1))
        qk_pool = top.enter_context(tc.tile_pool(name="qk", bufs=1))
        lat_pool = top.enter_context(tc.tile_pool(name="lat", bufs=1))
        uw_pool = top.enter_context(tc.tile_pool(name="uw", bufs=1))
        v_pool = top.enter_context(tc.tile_pool(name="v", bufs=1))

        bqf_t = const.tile([P, 2], f32)
        nc.sync.dma_start(bqf_t[:], bqf[:])
        bkvd_t = const.tile([P, LT], f32)
        nc.sync.dma_start(bkvd_t[:], bkvd.rearrange("(o p) -> p o", p=P))
        bku_t = const.tile([P, 2], f32)
        nc.sync.dma_start(bku_t[:], bku[:])
        ones_r = const.tile([P, P], f16)
        nc.sync.dma_start(ones_r[:], ones[:])

        attn_outT = ao_pool.tile([P, NHG, seq], f16)
        qT = qk_pool.tile([P, NHG, seq], f16)  # slots: 0=x1,1=x2,2,3=rope
        kT = qk_pool.tile([P, NHG, seq], f16)
        kv_latT = lat_pool.tile([P, LT, seq], f16)
        v_bf = v_pool.tile([P, ST, NHG * P], f16)  # token-major v

        # HAM warmup: back-to-back matmuls while the initial weight /
        # activation DMAs stream in, so the first real matmuls run at 2.4GHz.
        with tc.tile_pool(name="warm", bufs=1, space="PSUM") as warm_pool:
            wtiles = [warm_pool.tile([P, P], f32, tag=f"w{i}", name=f"warm{i}")
                      for i in range(4)]
            for i in range(32):
                nc.tensor.matmul(wtiles[i % 4][:], ones_r[:], ones_r[:],
                                 start=True, stop=True)

        # ---------------- phase D: kv_lat + k_up + fused q ----------------
        with tc.tile_pool(name="wd", bufs=1) as wd_pool, \
             tc.tile_pool(name="hst", bufs=2 * HT + 6) as hst_pool, \
             tc.tile_pool(name="ut", bufs=4) as ut_pool, \
             tc.tile_pool(name="psd", bufs=6, space="PSUM") as psd:
            wkvd_t = wd_pool.tile([P, HT, LAT], f16)
            fq_t = wd_pool.tile([P, HT, 2 * P], f16)
            cos_t = const.tile([P, seq], f16)
            sin_t = const.tile([P, seq], f16)
            wku_t = uw_pool.tile([P, LT, 2 * P], f16)
            wvu_t = uw_pool.tile([P, LT, NHG * P], f16)
            bvu_bc = uw_pool.tile([P, NHG * P], f32)
            wo_t = uw_pool.tile([P, NHG, H], f16)

            def load_hs(sc):
                ssl = slice(sc * SC, (sc + 1) * SC)
                hts = []
                for ht in range(HT):
                    t = hst_pool.tile([P, SC], f16, tag="hst")
                    nc.sync.dma_start(t[:], hsT[ht * P:(ht + 1) * P, ssl])
                    hts.append(t)
                return hts

            # DMA order: what each phase needs, just before it needs it.
            # hs/wkvd interleaved per h-tile so the first kv chain can start
            # chasing the DMA stream almost immediately; late weights
            # (wvu/wo) stream behind sc2/sc3.
            hts0 = []
            for ht in range(HT):
                t = hst_pool.tile([P, SC], f16, tag="hst", name="hs0")
                nc.sync.dma_start(t[:], hsT[ht * P:(ht + 1) * P, :SC])
                hts0.append(t)
                nc.sync.dma_start(
                    wkvd_t[:, ht, :], WkvdT[ht * P:(ht + 1) * P, :])
            hs_pend = [hts0]
            for ht in range(HT):
                nc.sync.dma_start(fq_t[:, ht, :], FqT[ht * P:(ht + 1) * P, :])
            nc.sync.dma_start(
                wku_t[:], WkuT.rearrange("(lt p) m -> p lt m", p=P))
            nc.sync.dma_start(cos_t[:], cosT[:])
            nc.sync.dma_start(sin_t[:], sinT[:])
            hs_pend.append(load_hs(1))

            def rope(dstT, ssl):
                """slots 2,3 of dstT from slots 0,1 (x1,x2) for chunk ssl."""
                x1 = dstT[:, 0, ssl]
                x2 = dstT[:, 1, ssl]
                t1 = ut_pool.tile([P, SC], f16, tag="ropetmp")
                t2 = ut_pool.tile([P, SC], f16, tag="ropetmp")
                nc.vector.tensor_mul(t1[:], x1, cos_t[:, ssl])
                nc.vector.tensor_mul(t2[:], x2, sin_t[:, ssl])
                nc.vector.tensor_sub(dstT[:, 2, ssl], t1[:], t2[:])
                t3 = ut_pool.tile([P, SC], f16, tag="ropetmp")
                t4 = ut_pool.tile([P, SC], f16, tag="ropetmp")
                nc.vector.tensor_mul(t3[:], x1, sin_t[:, ssl])
                nc.vector.tensor_mul(t4[:], x2, cos_t[:, ssl])
                nc.vector.tensor_add(dstT[:, 3, ssl], t3[:], t4[:])

            for sc in range(NSC):
                ssl = slice(sc * SC, (sc + 1) * SC)
                hts = hs_pend.pop(0)
                if sc + 2 < NSC:
                    hs_pend.append(load_hs(sc + 2))
                if sc == 0:
                    nc.sync.dma_start(
                        wvu_t[:], WvuT.rearrange("(lt p) m -> p lt m", p=P))
                    nc.sync.dma_start(
                        bvu_bc[:], bvu[:].to_broadcast((P, NHG * P)))
                elif sc == 1:
                    for ct in range(NHG):
                        nc.sync.dma_start(
                            wo_t[:, ct, :], WoT[ct * P:(ct + 1) * P, :])
                # kv_lat chains
                for lt in range(LT):
                    ps = psd.tile([P, SC], f32, tag="psd")
                    for ht in range(HT):
                        nc.tensor.matmul(
                            ps[:], wkvd_t[:, ht, lt * P:(lt + 1) * P],
                            hts[ht][:], start=(ht == 0), stop=(ht == HT - 1))
                    nc.scalar.activation(
                        kv_latT[:, lt, ssl], ps[:], Act.Identity,
                        bias=bkvd_t[:, lt:lt + 1])
                # fused q slots 0,1, then k_up slots 0,1
                for cs in range(2):
                    psq = psd.tile([P, SC], f32, tag="psd")
                    for ht in range(HT):
                        nc.tensor.matmul(
                            psq[:], fq_t[:, ht, cs * P:(cs + 1) * P],
                            hts[ht][:], start=(ht == 0), stop=(ht == HT - 1))
                    nc.scalar.activation(
                        qT[:, cs, ssl], psq[:], Act.Identity,
                        bias=bqf_t[:, cs:cs + 1])
                for ck in range(2):
                    psk = psd.tile([P, SC], f32, tag="psd")
                    for lt in range(LT):
                        nc.tensor.matmul(
                            psk[:], wku_t[:, lt, ck * P:(ck + 1) * P],
                            kv_latT[:, lt, ssl],
                            start=(lt == 0), stop=(lt == LT - 1))
                    nc.scalar.activation(
                        kT[:, ck, ssl], psk[:], Act.Identity,
                        bias=bku_t[:, ck:ck + 1])
                rope(qT, ssl)
                rope(kT, ssl)

        # ------- phase A+O: software-pipelined attention + o_proj --------
        with tc.tile_pool(name="exp", bufs=2) as exp_pool, \
             tc.tile_pool(name="pss", bufs=2, space="PSUM") as pss, \
             tc.tile_pool(name="psav", bufs=2, space="PSUM") as psav, \
             tc.tile_pool(name="fillps", bufs=2, space="PSUM") as fillps, \
             tc.tile_pool(name="att", bufs=3) as at_pool, \
             tc.tile_pool(name="ot", bufs=4) as ot_pool:

            def v_chain(st):
                def go():
                    ps = fillps.tile([P, NHG * P], f32, tag="fill",
                                     name="vps")
                    for lt in range(LT):
                        nc.tensor.matmul(
                            ps[:], kv_latT[:, lt, st * P:(st + 1) * P],
                            wvu_t[:, lt, :],
                            start=(lt == 0), stop=(lt == LT - 1))
                    nc.vector.tensor_tensor(
                        v_bf[:, st, :], ps[:], bvu_bc[:], Alu.add)
                return go

            def o_chain(qc, mt):
                def go():
                    qsl = slice(qc * SC, (qc + 1) * SC)
                    msl = slice(mt * P, (mt + 1) * P)
                    ps = fillps.tile([P, SC], f32, tag="fill", name="ops")
                    for ct in range(NHG):
                        nc.tensor.matmul(
                            ps[:], wo_t[:, ct, msl], attn_outT[:, ct, qsl],
                            start=(ct == 0), stop=(ct == NHG - 1))
                    ot = ot_pool.tile([P, SC], f16, tag="ot")
                    # DVE (gpsimd has no PSUM port; an ACT Identity here
                    # would force exp-table reloads on the scalar engine)
                    nc.vector.tensor_copy(ot[:], ps[:])
                    nc.sync.dma_start(outT[msl, qsl], ot[:])
                return go

            fill = [v_chain(st) for st in range(ST)]

            def emit_fill():
                if fill:
                    fill.pop(0)()

            def finish_head(ph):
                """denominator + reciprocal + normalize for a head."""
                pqc, phh, pexpt, ppav = ph
                pqsl = slice(pqc * SC, (pqc + 1) * SC)
                # single strided reduce over the 16 exp tiles (cheaper on
                # DVE than 15 chained adds: no acc re-reads)
                acc = at_pool.tile([P, SC], f16, tag="acc")
                with nc.allow_low_precision("f16 softmax denominator"):
                    nc.vector.tensor_reduce(
                        acc[:], pexpt.rearrange("p kt q -> p q kt"),
                        mybir.AxisListType.X, Alu.add)
                psm = fillps.tile([P, SC], f32, tag="fill", name="sums")
                nc.tensor.matmul(psm[:], ones_r[:], acc[:],
                                 start=True, stop=True)
                rec_bc = at_pool.tile([P, SC], f32, tag="rec_bc")
                nc.vector.reciprocal_approx_fast(rec_bc[:], psm[:])
                nc.vector.tensor_tensor(
                    attn_outT[:, phh, pqsl], ppav[:], rec_bc[:], Alu.mult)

            prev = None  # (qc, h, acc, pav)
            for qc in range(NSC):
                qsl = slice(qc * SC, (qc + 1) * SC)
                for h in range(NHG):
                    expt = exp_pool.tile([P, ST, SC], f16, tag="expt")
                    if prev is not None:
                        ppav = psav.tile([P, SC], f32, tag="av")
                    for kth in range(ST // 2):
                        # QK pair -> one 2-bank exp (paces the ACT chain)
                        ps = pss.tile([P, 2, SC], f32, tag="score")
                        for half in (0, 1):
                            kt = 2 * kth + half
                            nc.tensor.matmul(
                                ps[:, half, :],
                                kT[:, h, kt * P:(kt + 1) * P],
                                qT[:, h, qsl], start=True, stop=True)
                        nc.scalar.activation(
                            expt[:, 2 * kth:2 * kth + 2, :], ps[:],
                            Act.Exp, scale=INV_SQRT_HD)
                        # previous head's AV pair on PE underneath the exps
                        if prev is not None:
                            pexpt = prev[4]
                            for half in (0, 1):
                                kt = 2 * kth + half
                                nc.tensor.matmul(
                                    ppav[:],
                                    v_bf[:, kt, prev[1] * P:(prev[1] + 1) * P],
                                    pexpt[:, kt, :],
                                    start=(kt == 0), stop=(kt == ST - 1))
                        # fill work (v chains early, o_proj of prior chunks)
                        if prev is None:
                            emit_fill()
                            emit_fill()
                        elif kth % 2 == 1:
                            emit_fill()
                    if prev is not None:
                        finish_head((prev[0], prev[1], prev[4], ppav))
                        # queue o_proj(pqc) only AFTER its last head's
                        # normalize is emitted (else the in-order PE queue
                        # deadlocks on a not-yet-emitted sum matmul)
                        if prev[1] == NHG - 1:
                            fill.extend(
                                o_chain(prev[0], mt) for mt in range(HT))
                    prev = (qc, h, None, None, expt)

            # drain: AV + finish for the last head, then remaining fill
            ppav = psav.tile([P, SC], f32, tag="av")
            pexpt = prev[4]
            for kt in range(ST):
                nc.tensor.matmul(
                    ppav[:], v_bf[:, kt, prev[1] * P:(prev[1] + 1) * P],
                    pexpt[:, kt, :], start=(kt == 0), stop=(kt == ST - 1))
                if kt % 2 == 1:
                    emit_fill()
            finish_head((prev[0], prev[1], prev[4], ppav))
            fill.extend(o_chain(prev[0], mt) for mt in range(HT))
            while fill:
                emit_fill()

    nc.compile()
    return nc


# ---------------------------------------------------------------------------
# Host side: shard inputs, run SPMD, gather.
# ---------------------------------------------------------------------------

def _rope_cos_sin(seq_len, dim, base=10000.0):
    inv_freq = 1.0 / (base ** (np.arange(0, dim, 2, dtype=np.float32) / dim))
    t = np.arange(seq_len, dtype=np.float32)
    freqs = np.outer(t, inv_freq).astype(np.float32)
    emb = np.concatenate([freqs, freqs], -1)
    return np.cos(emb).astype(np.float32), np.sin(emb).astype(np.float32)


def make_in_maps(hidden_states, Wq_down, bq_down, Wkv_down, bkv_down,
                 Wq_up, bq_up, Wk_up, bk_up, Wv_up, bv_up, Wo, bo):
    cos, sin = _rope_cos_sin(S, ROPE_DIM)
    WkvdT = np.ascontiguousarray(Wkv_down.T).astype(F16)
    hsT = [np.ascontiguousarray(hidden_states[b].T).astype(F16)
           for b in range(B)]
    Wqd64 = Wq_down.astype(np.float64)
    bqd64 = bq_down.astype(np.float64)
    in_maps = []
    for c in range(8):
        b, j = c // 4, c % 4
        heads = [j, 4 + j, 8 + j, 12 + j]
        x1 = slice(j * P, (j + 1) * P)
        x2 = slice(512 + j * P, 512 + (j + 1) * P)
        vrows = np.concatenate(
            [np.arange(h * P, (h + 1) * P) for h in heads])
        Wqu_rows = np.concatenate([Wq_up[x1], Wq_up[x2]], 0).astype(
            np.float64)                       # [256, LAT]
        Fq = Wqu_rows @ Wqd64                 # [256, H]
        bq_fused = (Wqu_rows @ bqd64
                    + np.concatenate([bq_up[x1], bq_up[x2]]).astype(
                        np.float64))          # [256]
        in_maps.append(dict(
            hsT=hsT[b],
            FqT=np.ascontiguousarray(Fq.T).astype(F16),
            bqf=np.stack([bq_fused[:P], bq_fused[P:]], axis=1).astype(
                np.float32).copy(),
            WkvdT=WkvdT,
            bkvd=np.ascontiguousarray(bkv_down),
            WkuT=np.ascontiguousarray(
                np.concatenate([Wk_up[x1], Wk_up[x2]], 0).T).astype(F16),
            bku=np.stack([bk_up[x1], bk_up[x2]], axis=1).copy(),
            WvuT=np.ascontiguousarray(Wv_up[vrows].T).astype(F16),
            bvu=np.ascontiguousarray(bv_up[vrows][None, :]),
            WoT=np.ascontiguousarray(Wo[:, vrows].T).astype(F16),
            cosT=np.ascontiguousarray(cos[:, x1].T).astype(F16),
            sinT=np.ascontiguousarray(sin[:, x1].T).astype(F16),
            ones=np.ones((P, P), np.float16),
        ))
    return in_maps


_NC_CACHE = {}


def _get_nc():
    if "nc" not in _NC_CACHE:
        _NC_CACHE["nc"] = build_mla()
    return _NC_CACHE["nc"]


LAST_RESULTS = None  # BassKernelResults of the most recent kernel() call


def kernel(**inputs):
    global LAST_RESULTS
    nc = _get_nc()
    in_maps = make_in_maps(**inputs)
    trace = bool(int(os.environ.get("MLA_TRACE", "0")))
    kwargs = {}
    if trace:
        tc_env = os.environ.get("MLA_TRACE_CORES", "0,1,2,3,4,5,6,7")
        kwargs["trace_cores"] = [int(x) for x in tc_env.split(",")]
    res = run_bass_kernel_spmd(
        nc, in_maps, core_ids=list(range(8)), trace=trace, **kwargs)
    LAST_RESULTS = res
    bo = inputs["bo"]
    out = np.zeros((B, S, H), np.float32)
    for b in range(B):
        acc = res.results[b * 4]["outT"].astype(np.float32)
        for j in range(1, 4):
            acc = acc + res.results[b * 4 + j]["outT"]
        out[b] = acc.T + bo[None, :]
    return out


# revision 25
# speedup vs baseline: 1.4699x; 1.4699x over previous
"""Trainium2 Bass kernel for MultiHeadLatentAttention (B=2, S=2048, H=2048,
NH=16, HD=128, LAT=512), SPMD across 8 NeuronCores.

Sharding: 8 cores = 2 (batch) x 4 (head-group TP). Core c handles batch c//4
and head group j = c%4 = heads {j, 4+j, 8+j, 12+j} (slots 0..3). Heads j/4+j
are the raw x1/x2 slices of q_half; heads 8+j/12+j are their RoPE
combinations, so the up-projection shards 4-way with no duplication. Each
core computes its partial o_proj output (f32); the host sums the 4 partials
per batch and adds bo.

v3 structure:
- q path host-fused: Fq = Wq_up[core rows] @ Wq_down -> contract hidden
  directly into q slots 0/1 (kills the 4x-replicated q_down work).
- kv_lat f16; k_up runs inside the D loop per s-chunk (cheap, 4 matmuls per
  chain); ropes inline on DVE.
- phase A is a software-pipelined instruction stream: QK pairs (paced by the
  ACT exp chain) interleave with the PREVIOUS head's AV accumulation,
  denominator adds on DVE, and "fill" PE work (v-up chains first, then the
  previous q-chunk's o_proj) so the tensor engine never idles waiting for
  exps. o_proj results go psum -> DRAM f32 directly (no DVE copy).
"""
import os
import sys
import types
from contextlib import ExitStack

import numpy as np

if "/opt/trn_rl_repo" not in sys.path:
    sys.path.insert(0, "/opt/trn_rl_repo")

# ---------------------------------------------------------------------------
# NTFF-profile shim: antenv.axon_hooks is missing in this image; register a
# hook backed by the axon PJRT .so so trace=True can capture HW exec time.
# ---------------------------------------------------------------------------


def _install_axon_hooks_shim():
    if "antenv.axon_hooks" in sys.modules:
        return
    try:
        import antenv
        from trn_agent_boot.trn_boot import _ntff_profile_via_ctypes
        hook = _ntff_profile_via_ctypes("/opt/axon/libaxon_pjrt.so")
    except Exception:
        return
    mod = types.ModuleType("antenv.axon_hooks")
    mod.get_axon_ntff_profile_hook = lambda: hook
    mod.set_axon_ntff_profile_hook = lambda h: None
    sys.modules["antenv.axon_hooks"] = mod
    antenv.axon_hooks = mod


_install_axon_hooks_shim()

import concourse.bass as bass  # noqa: E402
import concourse.mybir as mybir  # noqa: E402
import concourse.tile as tile  # noqa: E402
from concourse import bacc  # noqa: E402
from concourse.bass_utils import run_bass_kernel_spmd  # noqa: E402

P = 128
H = 2048
NH = 16
HD = 128
LAT = 512
B = 2
S = 2048
ROPE_DIM = H // 4
NHG = 4          # heads per core
SC = 512         # s/q chunk (one PSUM bank of fp32)
INV_SQRT_HD = 0.08838834764831845  # 1/sqrt(128)

f32 = mybir.dt.float32
f16 = mybir.dt.float16
Act = mybir.ActivationFunctionType
Alu = mybir.AluOpType
F16 = np.float16


def build_mla(seq=S, debug=False):
    """Build one core's program. All cores run this same program SPMD."""
    NSC = seq // SC   # s-chunks
    HT = H // P       # 16 h-tiles
    LT = LAT // P     # 4 l-tiles
    ST = seq // P     # s-tiles (= k-tiles in attention)

    nc = bacc.Bacc("TRN2", target_bir_lowering=False, debug=debug)

    hsT = nc.dram_tensor("hsT", [H, seq], f16, kind="ExternalInput")
    FqT = nc.dram_tensor("FqT", [H, 2 * P], f16, kind="ExternalInput")
    bqf = nc.dram_tensor("bqf", [P, 2], f32, kind="ExternalInput")
    WkvdT = nc.dram_tensor("WkvdT", [H, LAT], f16, kind="ExternalInput")
    bkvd = nc.dram_tensor("bkvd", [LAT], f32, kind="ExternalInput")
    WkuT = nc.dram_tensor("WkuT", [LAT, 2 * P], f16, kind="ExternalInput")
    bku = nc.dram_tensor("bku", [P, 2], f32, kind="ExternalInput")
    WvuT = nc.dram_tensor("WvuT", [LAT, NHG * P], f16, kind="ExternalInput")
    bvu = nc.dram_tensor("bvu", [1, NHG * P], f32, kind="ExternalInput")
    WoT = nc.dram_tensor("WoT", [NHG * P, H], f16, kind="ExternalInput")
    cosT = nc.dram_tensor("cosT", [P, seq], f16, kind="ExternalInput")
    sinT = nc.dram_tensor("sinT", [P, seq], f16, kind="ExternalInput")
    ones = nc.dram_tensor("ones", [P, P], f16, kind="ExternalInput")
    outT = nc.dram_tensor("outT", [H, seq], f16, kind="ExternalOutput")

    with tile.TileContext(nc) as tc, ExitStack() as top:
        const = top.enter_context(tc.tile_pool(name="const", bufs=1))
        ao_pool = top.enter_context(tc.tile_pool(name="ao", bufs=1))
        qk_pool = top.enter_context(tc.tile_pool(name="qk", bufs=1))
        lat_pool = top.enter_context(tc.tile_pool(name="lat", bufs=1))
        uw_pool = top.enter_context(tc.tile_pool(name="uw", bufs=1))
        v_pool = top.enter_context(tc.tile_pool(name="v", bufs=1))

        bqf_t = const.tile([P, 2], f32)
        nc.sync.dma_start(bqf_t[:], bqf[:])
        bkvd_t = const.tile([P, LT], f32)
        nc.sync.dma_start(bkvd_t[:], bkvd.rearrange("(o p) -> p o", p=P))
        bku_t = const.tile([P, 2], f32)
        nc.sync.dma_start(bku_t[:], bku[:])
        ones_r = const.tile([P, P], f16)
        nc.sync.dma_start(ones_r[:], ones[:])

        attn_outT = ao_pool.tile([P, NHG, seq], f16)
        qT = qk_pool.tile([P, NHG, seq], f16)  # slots: 0=x1,1=x2,2,3=rope
        kT = qk_pool.tile([P, NHG, seq], f16)
        kv_latT = lat_pool.tile([P, LT, seq], f16)
        v_bf = v_pool.tile([P, ST, NHG * P], f16)  # token-major v

        # HAM warmup: back-to-back matmuls while the initial weight /
        # activation DMAs stream in, so the first real matmuls run at 2.4GHz.
        with tc.tile_pool(name="warm", bufs=1, space="PSUM") as warm_pool:
            wtiles = [warm_pool.tile([P, P], f32, tag=f"w{i}", name=f"warm{i}")
                      for i in range(4)]
            for i in range(32):
                nc.tensor.matmul(wtiles[i % 4][:], ones_r[:], ones_r[:],
                                 start=True, stop=True)

        # ---------------- phase D: kv_lat + k_up + fused q ----------------
        with tc.tile_pool(name="wd", bufs=1) as wd_pool, \
             tc.tile_pool(name="hst", bufs=2 * HT + 6) as hst_pool, \
             tc.tile_pool(name="ut", bufs=4) as ut_pool, \
             tc.tile_pool(name="psd", bufs=6, space="PSUM") as psd:
            wkvd_t = wd_pool.tile([P, HT, LAT], f16)
            fq_t = wd_pool.tile([P, HT, 2 * P], f16)
            cos_t = const.tile([P, seq], f16)
            sin_t = const.tile([P, seq], f16)
            wku_t = uw_pool.tile([P, LT, 2 * P], f16)
            wvu_t = uw_pool.tile([P, LT, NHG * P], f16)
            bvu_bc = uw_pool.tile([P, NHG * P], f32)
            wo_t = uw_pool.tile([P, NHG, H], f16)

            def load_hs(sc):
                ssl = slice(sc * SC, (sc + 1) * SC)
                hts = []
                for ht in range(HT):
                    t = hst_pool.tile([P, SC], f16, tag="hst")
                    nc.sync.dma_start(t[:], hsT[ht * P:(ht + 1) * P, ssl])
                    hts.append(t)
                return hts

            # DMA order: what each phase needs, just before it needs it.
            # hs/wkvd interleaved per h-tile so the first kv chain can start
            # chasing the DMA stream almost immediately; late weights
            # (wvu/wo) stream behind sc2/sc3.
            hts0 = []
            for ht in range(HT):
                t = hst_pool.tile([P, SC], f16, tag="hst", name="hs0")
                nc.sync.dma_start(t[:], hsT[ht * P:(ht + 1) * P, :SC])
                hts0.append(t)
                nc.sync.dma_start(
                    wkvd_t[:, ht, :], WkvdT[ht * P:(ht + 1) * P, :])
            hs_pend = [hts0]
            for ht in range(HT):
                nc.sync.dma_start(fq_t[:, ht, :], FqT[ht * P:(ht + 1) * P, :])
            nc.sync.dma_start(
                wku_t[:], WkuT.rearrange("(lt p) m -> p lt m", p=P))
            nc.sync.dma_start(cos_t[:], cosT[:])
            nc.sync.dma_start(sin_t[:], sinT[:])
            hs_pend.append(load_hs(1))

            def rope(dstT, ssl):
                """slots 2,3 of dstT from slots 0,1 (x1,x2) for chunk ssl."""
                x1 = dstT[:, 0, ssl]
                x2 = dstT[:, 1, ssl]
                t1 = ut_pool.tile([P, SC], f16, tag="ropetmp")
                t2 = ut_pool.tile([P, SC], f16, tag="ropetmp")
                nc.vector.tensor_mul(t1[:], x1, cos_t[:, ssl])
                nc.vector.tensor_mul(t2[:], x2, sin_t[:, ssl])
                nc.vector.tensor_sub(dstT[:, 2, ssl], t1[:], t2[:])
                t3 = ut_pool.tile([P, SC], f16, tag="ropetmp")
                t4 = ut_pool.tile([P, SC], f16, tag="ropetmp")
                nc.vector.tensor_mul(t3[:], x1, sin_t[:, ssl])
                nc.vector.tensor_mul(t4[:], x2, cos_t[:, ssl])
                nc.vector.tensor_add(dstT[:, 3, ssl], t3[:], t4[:])

            for sc in range(NSC):
                ssl = slice(sc * SC, (sc + 1) * SC)
                hts = hs_pend.pop(0)
                if sc + 2 < NSC:
                    hs_pend.append(load_hs(sc + 2))
                if sc == 0:
                    nc.sync.dma_start(
                        wvu_t[:], WvuT.rearrange("(lt p) m -> p lt m", p=P))
                    nc.sync.dma_start(
                        bvu_bc[:], bvu[:].to_broadcast((P, NHG * P)))
                elif sc == 1:
                    for ct in range(NHG):
                        nc.sync.dma_start(
                            wo_t[:, ct, :], WoT[ct * P:(ct + 1) * P, :])
                # kv_lat chains
                for lt in range(LT):
                    ps = psd.tile([P, SC], f32, tag="psd")
                    for ht in range(HT):
                        nc.tensor.matmul(
                            ps[:], wkvd_t[:, ht, lt * P:(lt + 1) * P],
                            hts[ht][:], start=(ht == 0), stop=(ht == HT - 1))
                    nc.scalar.activation(
                        kv_latT[:, lt, ssl], ps[:], Act.Identity,
                        bias=bkvd_t[:, lt:lt + 1])
                # fused q slots 0,1, then k_up slots 0,1
                for cs in range(2):
                    psq = psd.tile([P, SC], f32, tag="psd")
                    for ht in range(HT):
                        nc.tensor.matmul(
                            psq[:], fq_t[:, ht, cs * P:(cs + 1) * P],
                            hts[ht][:], start=(ht == 0), stop=(ht == HT - 1))
                    nc.scalar.activation(
                        qT[:, cs, ssl], psq[:], Act.Identity,
                        bias=bqf_t[:, cs:cs + 1])
                for ck in range(2):
                    psk = psd.tile([P, SC], f32, tag="psd")
                    for lt in range(LT):
                        nc.tensor.matmul(
                            psk[:], wku_t[:, lt, ck * P:(ck + 1) * P],
                            kv_latT[:, lt, ssl],
                            start=(lt == 0), stop=(lt == LT - 1))
                    nc.scalar.activation(
                        kT[:, ck, ssl], psk[:], Act.Identity,
                        bias=bku_t[:, ck:ck + 1])
                rope(qT, ssl)
                rope(kT, ssl)

        # ------- phase A+O: software-pipelined attention + o_proj --------
        with tc.tile_pool(name="exp", bufs=2) as exp_pool, \
             tc.tile_pool(name="pss", bufs=2, space="PSUM") as pss, \
             tc.tile_pool(name="psav", bufs=2, space="PSUM") as psav, \
             tc.tile_pool(name="fillps", bufs=2, space="PSUM") as fillps, \
             tc.tile_pool(name="att", bufs=3) as at_pool, \
             tc.tile_pool(name="ot", bufs=4) as ot_pool:

            def v_chain(st):
                def go():
                    ps = fillps.tile([P, NHG * P], f32, tag="fill",
                                     name="vps")
                    for lt in range(LT):
                        nc.tensor.matmul(
                            ps[:], kv_latT[:, lt, st * P:(st + 1) * P],
                            wvu_t[:, lt, :],
                            start=(lt == 0), stop=(lt == LT - 1))
                    nc.vector.tensor_tensor(
                        v_bf[:, st, :], ps[:], bvu_bc[:], Alu.add)
                return go

            def o_chain(qc, mt):
                def go():
                    qsl = slice(qc * SC, (qc + 1) * SC)
                    msl = slice(mt * P, (mt + 1) * P)
                    ps = fillps.tile([P, SC], f32, tag="fill", name="ops")
                    for ct in range(NHG):
                        nc.tensor.matmul(
                            ps[:], wo_t[:, ct, msl], attn_outT[:, ct, qsl],
                            start=(ct == 0), stop=(ct == NHG - 1))
                    ot = ot_pool.tile([P, SC], f16, tag="ot")
                    # DVE (gpsimd has no PSUM port; an ACT Identity here
                    # would force exp-table reloads on the scalar engine)
                    nc.vector.tensor_copy(ot[:], ps[:])
                    nc.sync.dma_start(outT[msl, qsl], ot[:])
                return go

            fill = [v_chain(st) for st in range(ST)]

            def emit_fill():
                if fill:
                    fill.pop(0)()

            def finish_head(ph):
                """denominator + reciprocal + normalize for a head."""
                pqc, phh, pexpt, ppav = ph
                pqsl = slice(pqc * SC, (pqc + 1) * SC)
                # contiguous half-split add tree over the 16 exp tiles:
                # 4 wide DVE ops instead of 15 chained adds (fewer
                # per-instruction overheads; a strided tensor_reduce is
                # 13us/op on HW — do NOT use it here)
                acc4 = at_pool.tile([P, 8, SC], f16, tag="acc4", bufs=2)
                nc.vector.tensor_tensor(
                    acc4[:], pexpt[:, 0:8, :], pexpt[:, 8:16, :], Alu.add)
                acc2 = at_pool.tile([P, 4, SC], f16, tag="acc2", bufs=2)
                nc.vector.tensor_tensor(
                    acc2[:], acc4[:, 0:4, :], acc4[:, 4:8, :], Alu.add)
                acc1 = at_pool.tile([P, 2, SC], f16, tag="acc1", bufs=2)
                nc.vector.tensor_tensor(
                    acc1[:], acc2[:, 0:2, :], acc2[:, 2:4, :], Alu.add)
                acc = at_pool.tile([P, SC], f16, tag="acc")
                nc.vector.tensor_tensor(
                    acc[:], acc1[:, 0, :], acc1[:, 1, :], Alu.add)
                psm = fillps.tile([P, SC], f32, tag="fill", name="sums")
                nc.tensor.matmul(psm[:], ones_r[:], acc[:],
                                 start=True, stop=True)
                rec_bc = at_pool.tile([P, SC], f32, tag="rec_bc")
                nc.vector.reciprocal_approx_fast(rec_bc[:], psm[:])
                nc.vector.tensor_tensor(
                    attn_outT[:, phh, pqsl], ppav[:], rec_bc[:], Alu.mult)

            prev = None  # (qc, h, acc, pav)
            for qc in range(NSC):
                qsl = slice(qc * SC, (qc + 1) * SC)
                for h in range(NHG):
                    expt = exp_pool.tile([P, ST, SC], f16, tag="expt")
                    if prev is not None:
                        ppav = psav.tile([P, SC], f32, tag="av")
                    for kth in range(ST // 2):
                        # QK pair -> one 2-bank exp (paces the ACT chain)
                        ps = pss.tile([P, 2, SC], f32, tag="score")
                        for half in (0, 1):
                            kt = 2 * kth + half
                            nc.tensor.matmul(
                                ps[:, half, :],
                                kT[:, h, kt * P:(kt + 1) * P],
                                qT[:, h, qsl], start=True, stop=True)
                        nc.scalar.activation(
                            expt[:, 2 * kth:2 * kth + 2, :], ps[:],
                            Act.Exp, scale=INV_SQRT_HD)
                        # previous head's AV pair on PE underneath the exps
                        if prev is not None:
                            pexpt = prev[4]
                            for half in (0, 1):
                                kt = 2 * kth + half
                                nc.tensor.matmul(
                                    ppav[:],
                                    v_bf[:, kt, prev[1] * P:(prev[1] + 1) * P],
                                    pexpt[:, kt, :],
                                    start=(kt == 0), stop=(kt == ST - 1))
                        # fill work (v chains early, o_proj of prior chunks)
                        if prev is None:
                            emit_fill()
                            emit_fill()
                        elif kth % 2 == 1:
                            emit_fill()
                    if prev is not None:
                        finish_head((prev[0], prev[1], prev[4], ppav))
                        # queue o_proj(pqc) only AFTER its last head's
                        # normalize is emitted (else the in-order PE queue
                        # deadlocks on a not-yet-emitted sum matmul)
                        if prev[1] == NHG - 1:
                            fill.extend(
                                o_chain(prev[0], mt) for mt in range(HT))
                    prev = (qc, h, None, None, expt)

            # drain: AV + finish for the last head, then remaining fill
            ppav = psav.tile([P, SC], f32, tag="av")
            pexpt = prev[4]
            for kt in range(ST):
                nc.tensor.matmul(
                    ppav[:], v_bf[:, kt, prev[1] * P:(prev[1] + 1) * P],
                    pexpt[:, kt, :], start=(kt == 0), stop=(kt == ST - 1))
                if kt % 2 == 1:
                    emit_fill()
            finish_head((prev[0], prev[1], prev[4], ppav))
            fill.extend(o_chain(prev[0], mt) for mt in range(HT))
            while fill:
                emit_fill()

    nc.compile()
    return nc


# ---------------------------------------------------------------------------
# Host side: shard inputs, run SPMD, gather.
# ---------------------------------------------------------------------------

def _rope_cos_sin(seq_len, dim, base=10000.0):
    inv_freq = 1.0 / (base ** (np.arange(0, dim, 2, dtype=np.float32) / dim))
    t = np.arange(seq_len, dtype=np.float32)
    freqs = np.outer(t, inv_freq).astype(np.float32)
    emb = np.concatenate([freqs, freqs], -1)
    return np.cos(emb).astype(np.float32), np.sin(emb).astype(np.float32)


def make_in_maps(hidden_states, Wq_down, bq_down, Wkv_down, bkv_down,
                 Wq_up, bq_up, Wk_up, bk_up, Wv_up, bv_up, Wo, bo):
    cos, sin = _rope_cos_sin(S, ROPE_DIM)
    WkvdT = np.ascontiguousarray(Wkv_down.T).astype(F16)
    hsT = [np.ascontiguousarray(hidden_states[b].T).astype(F16)
           for b in range(B)]
    Wqd64 = Wq_down.astype(np.float64)
    bqd64 = bq_down.astype(np.float64)
    in_maps = []
    for c in range(8):
        b, j = c // 4, c % 4
        heads = [j, 4 + j, 8 + j, 12 + j]
        x1 = slice(j * P, (j + 1) * P)
        x2 = slice(512 + j * P, 512 + (j + 1) * P)
        vrows = np.concatenate(
            [np.arange(h * P, (h + 1) * P) for h in heads])
        Wqu_rows = np.concatenate([Wq_up[x1], Wq_up[x2]], 0).astype(
            np.float64)                       # [256, LAT]
        Fq = Wqu_rows @ Wqd64                 # [256, H]
        bq_fused = (Wqu_rows @ bqd64
                    + np.concatenate([bq_up[x1], bq_up[x2]]).astype(
                        np.float64))          # [256]
        in_maps.append(dict(
            hsT=hsT[b],
            FqT=np.ascontiguousarray(Fq.T).astype(F16),
            bqf=np.stack([bq_fused[:P], bq_fused[P:]], axis=1).astype(
                np.float32).copy(),
            WkvdT=WkvdT,
            bkvd=np.ascontiguousarray(bkv_down),
            WkuT=np.ascontiguousarray(
                np.concatenate([Wk_up[x1], Wk_up[x2]], 0).T).astype(F16),
            bku=np.stack([bk_up[x1], bk_up[x2]], axis=1).copy(),
            WvuT=np.ascontiguousarray(Wv_up[vrows].T).astype(F16),
            bvu=np.ascontiguousarray(bv_up[vrows][None, :]),
            WoT=np.ascontiguousarray(Wo[:, vrows].T).astype(F16),
            cosT=np.ascontiguousarray(cos[:, x1].T).astype(F16),
            sinT=np.ascontiguousarray(sin[:, x1].T).astype(F16),
            ones=np.ones((P, P), np.float16),
        ))
    return in_maps


_NC_CACHE = {}


def _get_nc():
    if "nc" not in _NC_CACHE:
        _NC_CACHE["nc"] = build_mla()
    return _NC_CACHE["nc"]


LAST_RESULTS = None  # BassKernelResults of the most recent kernel() call


def kernel(**inputs):
    global LAST_RESULTS
    nc = _get_nc()
    in_maps = make_in_maps(**inputs)
    trace = bool(int(os.environ.get("MLA_TRACE", "0")))
    kwargs = {}
    if trace:
        tc_env = os.environ.get("MLA_TRACE_CORES", "0,1,2,3,4,5,6,7")
        kwargs["trace_cores"] = [int(x) for x in tc_env.split(",")]
    res = run_bass_kernel_spmd(
        nc, in_maps, core_ids=list(range(8)), trace=trace, **kwargs)
    LAST_RESULTS = res
    bo = inputs["bo"]
    out = np.zeros((B, S, H), np.float32)
    for b in range(B):
        acc = res.results[b * 4]["outT"].astype(np.float32)
        for j in range(1, 4):
            acc = acc + res.results[b * 4 + j]["outT"]
        out[b] = acc.T + bo[None, :]
    return out
